# revision 2
# baseline (speedup 1.0000x reference)
"""Trainium2 Bass kernel for nn_AttentionModel (B=4,S=2048,H=8,E=64, dropout mask).

Strategy: shard the 32 (b,h) pairs over 8 cores (4 pairs/core). All device
compute is done in the *transposed* orientation scoresT[t,s] so that the PV
matmul consumes probsT directly with no big on-chip transposes:

  qTproj[f,s] = Wq_aug.T @ qT_aug      (K=65 rows: 64 e-rows + ones/bias row)
  scoresT[t,s] = kTproj[:,t].T @ qTproj[:,s]     (K=64)
  expT = exp(scoresT/8)  (ACT, PSUM->SBUF, bf16)
  den[s] = ones.T @ expT                (PE ones-matmul, accumulated fp32)
  probsT = expT * maskT                 (DVE bf16 2x mode)
  outT[e,s] += vproj[t,:].T @ probsT    (PE, bf16)
  out[s,e] = transpose(outT) * (1/(0.9*den[s]))   (PE transpose + DVE scale)

Host side only does layout prep (transposes / bf16 cast / shard / gather).
"""

import os
import sys

sys.path.insert(0, "/opt/trn_rl_repo")

import ml_dtypes
import numpy as np

import concourse.bass as bass
import concourse.mybir as mybir
import concourse.tile as tile
from concourse import bacc, bass_utils
from concourse.bass import ds, ts
from concourse.masks import make_identity

B, S, H, E = 4, 2048, 8, 64
NCORES = 8
PAIRS = (B * H) // NCORES  # 4 (b,h) pairs per core
SC = 1024                  # s-chunk width for the streaming loop
NSC = S // SC              # 2
NTT = S // 128             # 16 t-tiles
F32 = mybir.dt.float32
BF16 = mybir.dt.bfloat16
INV_KEEP = 1.0 / 0.9

_CACHED_NC = None


def _body(tc, qT_d, kT_d, vT_d, mT_d, wq_d, bq_d, wk_d, bk_d, wv_d, bv_d, out_d):
    nc = tc.nc
    Exp = mybir.ActivationFunctionType.Exp
    with (
        tc.tile_pool(name="const", bufs=1) as const,
        tc.tile_pool(name="io", bufs=2) as io,
        tc.tile_pool(name="proj", bufs=2) as proj,
        tc.tile_pool(name="work", bufs=3) as work,
        tc.tile_pool(name="psA", bufs=2, space=bass.MemorySpace.PSUM) as psA,
        tc.tile_pool(name="psB", bufs=1, space=bass.MemorySpace.PSUM) as psB,
        tc.tile_pool(name="psD", bufs=1, space=bass.MemorySpace.PSUM) as psD,
    ):
        # --- constants ---
        wq = const.tile([E + 1, E], F32, tag="wq")
        wk = const.tile([E + 1, E], F32, tag="wk")
        wv = const.tile([E + 1, E], F32, tag="wv")
        nc.sync.dma_start(wq[0:E, :], wq_d[:, :])
        nc.sync.dma_start(wq[E : E + 1, :], bq_d[:, :])
        nc.sync.dma_start(wk[0:E, :], wk_d[:, :])
        nc.sync.dma_start(wk[E : E + 1, :], bk_d[:, :])
        nc.sync.dma_start(wv[0:E, :], wv_d[:, :])
        nc.sync.dma_start(wv[E : E + 1, :], bv_d[:, :])
        ident = const.tile([E, E], F32, tag="ident")
        make_identity(nc, ident[:, :])
        ones = const.tile([128, 1], BF16, tag="ones")
        nc.vector.memset(ones[:, :], 1.0)
        zbias = const.tile([128, 1], F32, tag="zbias")
        nc.vector.memset(zbias[:, :], 0.0)

        for p in range(PAIRS):
            # --- stage inputs (augmented with a ones row for in-matmul bias) ---
            qt = io.tile([E + 1, S], F32, tag="qt")
            kt = io.tile([E + 1, S], F32, tag="kt")
            vt = io.tile([E + 1, S], F32, tag="vt")
            nc.sync.dma_start(qt[0:E, :], qT_d[p])
            nc.sync.dma_start(kt[0:E, :], kT_d[p])
            nc.sync.dma_start(vt[0:E, :], vT_d[p])
            nc.vector.memset(qt[E : E + 1, :], 1.0)
            nc.vector.memset(kt[E : E + 1, :], 1.0)
            nc.vector.memset(vt[E : E + 1, :], 1.0)

            # --- projections ---
            qp = proj.tile([E, S], F32, tag="qp")
            kp = proj.tile([E, S], F32, tag="kp")
            vp = proj.tile([128, NTT * E], BF16, tag="vp")
            for c in range(S // 512):
                pq = psA.tile([E, 512], F32, tag="scores")
                nc.tensor.matmul(pq[:, :], wq[:, :], qt[:, ts(c, 512)],
                                 start=True, stop=True)
                nc.vector.tensor_copy(qp[:, ts(c, 512)], pq[:, :])
                pk = psA.tile([E, 512], F32, tag="scores")
                nc.tensor.matmul(pk[:, :], wk[:, :], kt[:, ts(c, 512)],
                                 start=True, stop=True)
                nc.vector.tensor_copy(kp[:, ts(c, 512)], pk[:, :])
            for t in range(NTT):
                pv_ = psA.tile([128, E], F32, tag="scores")
                nc.tensor.matmul(pv_[:, :], vt[:, ts(t, 128)], wv[:, :],
                                 start=True, stop=True)
                nc.vector.tensor_copy(vp[:, ts(t, E)], pv_[:, :])

            # --- main streaming loop ---
            for c in range(NSC):
                den = psD.tile([1, SC], F32, tag="den")
                pvp = psB.tile([E, SC], F32, tag="pv")
                for t in range(NTT):
                    sp = psA.tile([128, SC], F32, tag="scores")
                    nc.tensor.matmul(sp[:, 0:512], kp[:, ts(t, 128)],
                                     qp[:, ds(c * SC, 512)],
                                     start=True, stop=True)
                    nc.tensor.matmul(sp[:, 512:1024], kp[:, ts(t, 128)],
                                     qp[:, ds(c * SC + 512, 512)],
                                     start=True, stop=True)
                    ex = work.tile([128, SC], BF16, tag="ex")
                    nc.scalar.activation(ex[:, :], sp[:, :], Exp,
                                         bias=zbias[:, :], scale=0.125)
                    nc.tensor.matmul(den[:, 0:512], ones[:, :], ex[:, 0:512],
                                     start=(t == 0), stop=(t == NTT - 1))
                    nc.tensor.matmul(den[:, 512:1024], ones[:, :], ex[:, 512:1024],
                                     start=(t == 0), stop=(t == NTT - 1))
                    mk = work.tile([128, SC], BF16, tag="mk")
                    nc.sync.dma_start(mk[:, :], mT_d[p, ts(t, 128), ds(c * SC, SC)])
                    pr = work.tile([128, SC], BF16, tag="pr")
                    nc.vector.tensor_mul(pr[:, :], ex[:, :], mk[:, :])
                    nc.tensor.matmul(pvp[:, 0:512], vp[:, ts(t, E)], pr[:, 0:512],
                                     start=(t == 0), stop=(t == NTT - 1))
                    nc.tensor.matmul(pvp[:, 512:1024], vp[:, ts(t, E)],
                                     pr[:, 512:1024],
                                     start=(t == 0), stop=(t == NTT - 1))
                # --- finalize this s-chunk ---
                drow = work.tile([1, SC], F32, tag="drow")
                nc.vector.tensor_copy(drow[:, :], den[:, :])
                dcol = work.tile([128, SC // 128], F32, tag="dcol")
                for i in range(SC // 128):
                    nc.sync.dma_start(dcol[:, i : i + 1], drow[0:1, ts(i, 128)])
                inv = work.tile([128, SC // 128], F32, tag="inv")
                nc.vector.reciprocal(inv[:, :], dcol[:, :])
                nc.vector.tensor_scalar_mul(inv[:, :], inv[:, :], INV_KEEP)
                pvs = work.tile([E, SC], F32, tag="pvs")
                nc.vector.tensor_copy(pvs[:, :], pvp[:, :])
                for st in range(SC // 128):
                    tp = psA.tile([128, E], F32, tag="scores")
                    nc.tensor.transpose(tp[:, :], pvs[:, ts(st, 128)], ident[:, :])
                    ot = work.tile([128, E], F32, tag="ot")
                    nc.vector.tensor_scalar_mul(ot[:, :], tp[:, :], inv[:, st : st + 1])
                    nc.sync.dma_start(out_d[p, ds(c * SC + st * 128, 128), :], ot[:, :])


def _build():
    global _CACHED_NC
    if _CACHED_NC is not None:
        return _CACHED_NC
    nc = bacc.Bacc("TRN2", target_bir_lowering=False, debug=False,
                   num_devices=NCORES)
    qT_d = nc.dram_tensor("qT", [PAIRS, E, S], F32, kind="ExternalInput").ap()
    kT_d = nc.dram_tensor("kT", [PAIRS, E, S], F32, kind="ExternalInput").ap()
    vT_d = nc.dram_tensor("vT", [PAIRS, E, S], F32, kind="ExternalInput").ap()
    mT_d = nc.dram_tensor("maskT", [PAIRS, S, S], BF16, kind="ExternalInput").ap()
    wq_d = nc.dram_tensor("Wq", [E, E], F32, kind="ExternalInput").ap()
    bq_d = nc.dram_tensor("bq", [1, E], F32, kind="ExternalInput").ap()
    wk_d = nc.dram_tensor("Wk", [E, E], F32, kind="ExternalInput").ap()
    bk_d = nc.dram_tensor("bk", [1, E], F32, kind="ExternalInput").ap()
    wv_d = nc.dram_tensor("Wv", [E, E], F32, kind="ExternalInput").ap()
    bv_d = nc.dram_tensor("bv", [1, E], F32, kind="ExternalInput").ap()
    out_d = nc.dram_tensor("out", [PAIRS, S, E], F32, kind="ExternalOutput").ap()
    with tile.TileContext(nc) as tc:
        _body(tc, qT_d, kT_d, vT_d, mT_d, wq_d, bq_d, wk_d, bk_d, wv_d, bv_d, out_d)
    nc.compile()
    _CACHED_NC = nc
    return nc


def _in_maps(inputs):
    query = np.asarray(inputs["query"], np.float32)
    key = np.asarray(inputs["key"], np.float32)
    value = np.asarray(inputs["value"], np.float32)
    mask = np.asarray(inputs["drop_mask"])
    # [B,S,H,E] -> [B*H, E, S]
    qT = np.ascontiguousarray(query.transpose(0, 2, 3, 1)).reshape(B * H, E, S)
    kT = np.ascontiguousarray(key.transpose(0, 2, 3, 1)).reshape(B * H, E, S)
    vT = np.ascontiguousarray(value.transpose(0, 2, 3, 1)).reshape(B * H, E, S)
    # [B,H,S,S] -> transposed [B*H, t, s] as bf16 {0,1}
    mT = (np.ascontiguousarray(mask.transpose(0, 1, 3, 2))
          .astype(ml_dtypes.bfloat16).reshape(B * H, S, S))
    Wq = np.asarray(inputs["Wq"], np.float32)
    Wk = np.asarray(inputs["Wk"], np.float32)
    Wv = np.asarray(inputs["Wv"], np.float32)
    bq = np.asarray(inputs["bq"], np.float32).reshape(1, E)
    bk = np.asarray(inputs["bk"], np.float32).reshape(1, E)
    bv = np.asarray(inputs["bv"], np.float32).reshape(1, E)
    maps = []
    for c in range(NCORES):
        sl = slice(c * PAIRS, (c + 1) * PAIRS)
        maps.append({
            "qT": np.ascontiguousarray(qT[sl]),
            "kT": np.ascontiguousarray(kT[sl]),
            "vT": np.ascontiguousarray(vT[sl]),
            "maskT": np.ascontiguousarray(mT[sl]),
            "Wq": Wq, "bq": bq, "Wk": Wk, "bk": bk, "Wv": Wv, "bv": bv,
        })
    return maps


def _gather(results):
    outs = [results[c]["out"] for c in range(NCORES)]
    return (np.concatenate(outs, axis=0)
            .reshape(B, H, S, E).astype(np.float32, copy=False))


def kernel(**inputs):
    nc = _build()
    maps = _in_maps(inputs)
    res = bass_utils.run_bass_kernel_spmd(nc, maps, core_ids=list(range(NCORES)))
    return _gather(res.results)


if __name__ == "__main__":
    _build()
    print("build+compile OK")


# revision 4
# speedup vs baseline: 2.0276x; 2.0276x over previous
"""Trainium2 Bass kernel for nn_AttentionModel (B=4,S=2048,H=8,E=64, dropout mask).

Sharding: the 32 (b,h) pairs over 8 cores (4 pairs/core). All device compute is
in the *transposed* orientation scoresT[t,s] so the PV matmul consumes probsT
directly with no big on-chip transposes:

  qTproj[f,s] = Wq_aug.T @ qT_aug      (K=65: 64 e-rows + host-appended ones row)
  scoresT[t,s] = kTproj[:,t].T @ qTproj[:,s]     (K=64, fp16)
  expT = exp(scoresT/8)  (ACT, PSUM->SBUF, fp16)
  den[s] = ones.T @ expT                (PE ones-matmul, fp32 accum)
  probsT = expT * maskT                 (DVE fp16 2x mode)
  outT[e,s] += vproj[t,:].T @ probsT    (PE, fp16)
  out[s,e] = transpose(outT) * (1/(0.9*den[s]))   (PE transpose + DVE scale)

den/PV run DEPTH iterations behind scores/exp (software pipeline) so the PE
FIFO never stalls waiting on ACT/DVE. Host side only does layout prep
(transpose / fp16 cast / shard / gather).
"""

import os
import sys

sys.path.insert(0, "/opt/trn_rl_repo")

import numpy as np

import concourse.bass as bass
import concourse.mybir as mybir
import concourse.tile as tile
from concourse import bacc, bass_utils
from concourse.bass import ds, ts
from concourse.masks import make_identity

B, S, H, E = 4, 2048, 8, 64
E1 = E + 1                 # augmented contraction (ones/bias row)
NCORES = 8
PAIRS = (B * H) // NCORES  # 4 (b,h) pairs per core
SC = 1024                  # s-chunk width
NSC = S // SC              # 2
NTT = S // 128             # 16 t-tiles
DEPTH = 2                  # den/pv pipeline delay (iterations)
F32 = mybir.dt.float32
FP16 = mybir.dt.float16
INV_KEEP = 1.0 / 0.9

_CACHED_NC = None


def _body(tc, qT_d, kT_d, vT_d, mT_d, wq_d, wk_d, wv_d, out_d):
    nc = tc.nc
    Exp = mybir.ActivationFunctionType.Exp
    with (
        tc.tile_pool(name="const", bufs=1) as const,
        tc.tile_pool(name="io", bufs=2) as io,
        tc.tile_pool(name="proj", bufs=2) as proj,
        tc.tile_pool(name="work", bufs=2 + DEPTH) as work,
        tc.tile_pool(name="fin", bufs=2) as fin,
        tc.tile_pool(name="psA", bufs=2, space=bass.MemorySpace.PSUM) as psA,
        tc.tile_pool(name="psB", bufs=1, space=bass.MemorySpace.PSUM) as psB,
        tc.tile_pool(name="psD", bufs=1, space=bass.MemorySpace.PSUM) as psD,
    ):
        # --- constants ---
        wq = const.tile([E1, E], FP16, tag="wq")
        wk = const.tile([E1, E], FP16, tag="wk")
        wv = const.tile([E1, E], FP16, tag="wv")
        nc.sync.dma_start(wq[:, :], wq_d[:, :])
        nc.sync.dma_start(wk[:, :], wk_d[:, :])
        nc.sync.dma_start(wv[:, :], wv_d[:, :])
        ident = const.tile([E, E], F32, tag="ident")
        make_identity(nc, ident[:, :])
        ones = const.tile([128, 1], FP16, tag="ones")
        nc.vector.memset(ones[:, :], 1.0)
        zbias = const.tile([128, 1], F32, tag="zbias")
        nc.vector.memset(zbias[:, :], 0.0)

        for p in range(PAIRS):
            # --- stage inputs (ones row appended host-side) ---
            qt = io.tile([E1, S], FP16, tag="qt")
            kt = io.tile([E1, S], FP16, tag="kt")
            vt = io.tile([E1, S], FP16, tag="vt")
            nc.sync.dma_start(qt[:, :], qT_d[p])
            nc.sync.dma_start(kt[:, :], kT_d[p])
            nc.sync.dma_start(vt[:, :], vT_d[p])

            # --- projections ---
            qp = proj.tile([E, S], FP16, tag="qp")
            kp = proj.tile([E, S], FP16, tag="kp")
            vp = proj.tile([128, NTT * E], FP16, tag="vp")
            for c in range(S // 512):
                pq = psA.tile([E, 512], F32, tag="scores")
                nc.tensor.matmul(pq[:, :], wq[:, :], qt[:, ts(c, 512)],
                                 start=True, stop=True)
                nc.vector.tensor_copy(qp[:, ts(c, 512)], pq[:, :])
                pk = psA.tile([E, 512], F32, tag="scores")
                nc.tensor.matmul(pk[:, :], wk[:, :], kt[:, ts(c, 512)],
                                 start=True, stop=True)
                nc.vector.tensor_copy(kp[:, ts(c, 512)], pk[:, :])
            for t in range(NTT):
                pv_ = psA.tile([128, E], F32, tag="scores")
                nc.tensor.matmul(pv_[:, :], vt[:, ts(t, 128)], wv[:, :],
                                 start=True, stop=True)
                nc.vector.tensor_copy(vp[:, ts(t, E)], pv_[:, :])

            # --- main streaming loop, den/pv delayed by DEPTH iterations ---
            steps = [(c, t) for c in range(NSC) for t in range(NTT)]
            exs, prs, dens, pvps = {}, {}, {}, {}

            def finalize(c):
                den, pvp = dens[c], pvps[c]
                drow = fin.tile([1, SC], F32, tag="drow")
                nc.vector.tensor_copy(drow[:, :], den[:, :])
                dcol = fin.tile([128, SC // 128], F32, tag="dcol")
                for i in range(SC // 128):
                    nc.sync.dma_start(dcol[:, i : i + 1], drow[0:1, ts(i, 128)])
                inv = fin.tile([128, SC // 128], F32, tag="inv")
                nc.vector.reciprocal(inv[:, :], dcol[:, :])
                nc.vector.tensor_scalar_mul(inv[:, :], inv[:, :], INV_KEEP)
                pvs = fin.tile([E, SC], F32, tag="pvs")
                nc.vector.tensor_copy(pvs[:, :], pvp[:, :])
                for st in range(SC // 128):
                    tp = psA.tile([128, E], F32, tag="scores")
                    nc.tensor.transpose(tp[:, :], pvs[:, ts(st, 128)], ident[:, :])
                    ot = fin.tile([128, E], F32, tag="ot")
                    nc.vector.tensor_scalar_mul(ot[:, :], tp[:, :],
                                                inv[:, st : st + 1])
                    nc.sync.dma_start(out_d[p, ds(c * SC + st * 128, 128), :],
                                      ot[:, :])

            for idx in range(len(steps) + DEPTH):
                if idx < len(steps):
                    c, t = steps[idx]
                    if t == 0:
                        dens[c] = psD.tile([1, SC], F32, tag="den", name="den")
                        pvps[c] = psB.tile([E, SC], F32, tag="pv", name="pvp")
                    sp = psA.tile([128, SC], F32, tag="scores")
                    nc.tensor.matmul(sp[:, 0:512], kp[:, ts(t, 128)],
                                     qp[:, ds(c * SC, 512)],
                                     start=True, stop=True)
                    nc.tensor.matmul(sp[:, 512:1024], kp[:, ts(t, 128)],
                                     qp[:, ds(c * SC + 512, 512)],
                                     start=True, stop=True)
                    ex = work.tile([128, SC], FP16, tag="ex")
                    nc.scalar.activation(ex[:, :], sp[:, :], Exp,
                                         bias=zbias[:, :], scale=0.125)
                    mk = work.tile([128, SC], FP16, tag="mk")
                    nc.sync.dma_start(mk[:, :], mT_d[p, ts(t, 128), ds(c * SC, SC)])
                    pr = work.tile([128, SC], FP16, tag="pr")
                    nc.vector.tensor_mul(pr[:, :], ex[:, :], mk[:, :])
                    exs[idx], prs[idx] = ex, pr
                if idx >= DEPTH:
                    c, t = steps[idx - DEPTH]
                    ex, pr = exs.pop(idx - DEPTH), prs.pop(idx - DEPTH)
                    den, pvp = dens[c], pvps[c]
                    nc.tensor.matmul(den[:, 0:512], ones[:, :], ex[:, 0:512],
                                     start=(t == 0), stop=(t == NTT - 1))
                    nc.tensor.matmul(den[:, 512:1024], ones[:, :],
                                     ex[:, 512:1024],
                                     start=(t == 0), stop=(t == NTT - 1))
                    nc.tensor.matmul(pvp[:, 0:512], vp[:, ts(t, E)],
                                     pr[:, 0:512],
                                     start=(t == 0), stop=(t == NTT - 1))
                    nc.tensor.matmul(pvp[:, 512:1024], vp[:, ts(t, E)],
                                     pr[:, 512:1024],
                                     start=(t == 0), stop=(t == NTT - 1))
                    if t == NTT - 1:
                        finalize(c)


def _build():
    global _CACHED_NC
    if _CACHED_NC is not None:
        return _CACHED_NC
    nc = bacc.Bacc("TRN2", target_bir_lowering=False, debug=False,
                   num_devices=NCORES)
    qT_d = nc.dram_tensor("qT", [PAIRS, E1, S], FP16, kind="ExternalInput").ap()
    kT_d = nc.dram_tensor("kT", [PAIRS, E1, S], FP16, kind="ExternalInput").ap()
    vT_d = nc.dram_tensor("vT", [PAIRS, E1, S], FP16, kind="ExternalInput").ap()
    mT_d = nc.dram_tensor("maskT", [PAIRS, S, S], FP16, kind="ExternalInput").ap()
    wq_d = nc.dram_tensor("Wq", [E1, E], FP16, kind="ExternalInput").ap()
    wk_d = nc.dram_tensor("Wk", [E1, E], FP16, kind="ExternalInput").ap()
    wv_d = nc.dram_tensor("Wv", [E1, E], FP16, kind="ExternalInput").ap()
    out_d = nc.dram_tensor("out", [PAIRS, S, E], F32, kind="ExternalOutput").ap()
    with tile.TileContext(nc) as tc:
        _body(tc, qT_d, kT_d, vT_d, mT_d, wq_d, wk_d, wv_d, out_d)
    nc.compile()
    _CACHED_NC = nc
    return nc


def _aug(xT):
    """[n, E, S] -> [n, E+1, S] fp16 with a ones row appended."""
    n = xT.shape[0]
    out = np.empty((n, E1, S), np.float16)
    out[:, :E, :] = xT
    out[:, E, :] = 1.0
    return out


def _in_maps(inputs):
    query = np.asarray(inputs["query"], np.float32)
    key = np.asarray(inputs["key"], np.float32)
    value = np.asarray(inputs["value"], np.float32)
    mask = np.asarray(inputs["drop_mask"])
    # [B,S,H,E] -> [B*H, E, S], fp16, + ones row
    qT = _aug(query.transpose(0, 2, 3, 1).reshape(B * H, E, S))
    kT = _aug(key.transpose(0, 2, 3, 1).reshape(B * H, E, S))
    vT = _aug(value.transpose(0, 2, 3, 1).reshape(B * H, E, S))
    # [B,H,S,S] -> transposed [B*H, t, s] as fp16 {0,1}
    mT = (np.ascontiguousarray(mask.transpose(0, 1, 3, 2))
          .astype(np.float16).reshape(B * H, S, S))

    def waug(W, b):
        out = np.empty((E1, E), np.float16)
        out[:E, :] = np.asarray(W, np.float32)
        out[E, :] = np.asarray(b, np.float32).reshape(E)
        return out

    Wq = waug(inputs["Wq"], inputs["bq"])
    Wk = waug(inputs["Wk"], inputs["bk"])
    Wv = waug(inputs["Wv"], inputs["bv"])
    maps = []
    for c in range(NCORES):
        sl = slice(c * PAIRS, (c + 1) * PAIRS)
        maps.append({
            "qT": np.ascontiguousarray(qT[sl]),
            "kT": np.ascontiguousarray(kT[sl]),
            "vT": np.ascontiguousarray(vT[sl]),
            "maskT": np.ascontiguousarray(mT[sl]),
            "Wq": Wq, "Wk": Wk, "Wv": Wv,
        })
    return maps


def _gather(results):
    outs = [results[c]["out"] for c in range(NCORES)]
    return (np.concatenate(outs, axis=0)
            .reshape(B, H, S, E).astype(np.float32, copy=False))


def kernel(**inputs):
    nc = _build()
    maps = _in_maps(inputs)
    res = bass_utils.run_bass_kernel_spmd(nc, maps, core_ids=list(range(NCORES)))
    return _gather(res.results)


if __name__ == "__main__":
    _build()
    print("build+compile OK")
